# revision 5
# baseline (speedup 1.0000x reference)
"""Attention w/ KV cache on 8 trn2 NeuronCores.

Sharding: core i handles (batch b = i//2, query-half = i%2).  Each core
computes full attention for 512 query rows of one batch against that
batch's 5120 keys (4096 cache + 1024 new).  No collectives: the output
projection contracts over all heads, which every core has for its rows.

All tensors are kept feature-major ("transposed") on chip so that every
matmul consumes operands in its natural orientation; the required
transposes (x^T, cache-K^T, partition-major V) are done host-side in
numpy inside kernel().  Matmuls run in float32r (full PE rate for free
dim >= 256), accumulation in fp32 PSUM.
"""

import sys

sys.path.insert(0, "/opt/trn_rl_repo")

import numpy as np

B, N, C, H, D = 4, 1024, 1024, 16, 64
LC = 4096          # cached keys
L = LC + N         # total keys
NH = N // 2        # query rows per core
NCORES = 8
SCALE = float(D) ** -0.5
CT = C // 128      # contraction tiles (8)
QM = C // 128      # q output col-tiles (8)
NT = N // 128      # new-token l-tiles (8)
LT = L // 128      # total l-tiles (40)
KCT = LC // 128    # cached l-tiles (32)

_CACHE = {}


def _build():
    import concourse.mybir as mybir
    import concourse.tile as tile
    from concourse import bacc
    from contextlib import ExitStack

    f32 = mybir.dt.float32
    f32r = mybir.dt.float32r
    AF = mybir.ActivationFunctionType
    MUL = mybir.AluOpType.mult

    nc = bacc.Bacc("TRN2", target_bir_lowering=False, debug=False)

    xT_d = nc.dram_tensor("xT", [C, N], f32r, kind="ExternalInput").ap()
    kT_d = nc.dram_tensor("kT", [H, 2, D, LC // 2], f32r, kind="ExternalInput").ap()
    v_d = nc.dram_tensor("v", [H, 128, D + 1, KCT], f32r, kind="ExternalInput").ap()
    wqkv_d = nc.dram_tensor("wqkv", [C, 3 * C], f32r, kind="ExternalInput").ap()
    wproj_d = nc.dram_tensor("wproj", [C, C], f32r, kind="ExternalInput").ap()
    bias_d = nc.dram_tensor("bias", [128, QM], f32, kind="ExternalInput").ap()
    yT_d = nc.dram_tensor("yT", [C, NH], f32, kind="ExternalOutput").ap()

    with tile.TileContext(nc) as tc, ExitStack() as es:
        pp = es.enter_context(tc.tile_pool(name="persist", bufs=1))
        qT = pp.tile([128, QM, NH], f32r)            # q^T, head h at (64*(h%2), h//2)
        qd2 = pp.tile([128, QM, NH], f32r)           # q^T copies on opposite halves
        kTn = pp.tile([128, QM, N], f32r)            # new-K^T, same packing
        vn = pp.tile([128, NT, H, D + 1], f32r)      # new-V + ones col, l-partition-major
        aoT = pp.tile([128, CT, NH], f32r)           # normalized attention out^T
        ones = pp.tile([1, 64], f32r)
        bias = pp.tile([128, QM], f32)
        ones32 = pp.tile([128, 128], f32)

        nc.vector.memset(ones32[:], 1.0)
        nc.vector.tensor_copy(ones[:], ones32[0:1, 0:64])
        nc.vector.tensor_copy(
            vn[:, :, :, D], ones32[:].rearrange("p (a b) -> p a b", a=NT)
        )
        nc.sync.dma_start(bias[:], bias_d[:])

        # ---------------- phase 1: projections ----------------
        with tc.tile_pool(name="ph1", bufs=1) as p1, \
             tc.tile_pool(name="w1", bufs=3) as wpool, \
             tc.tile_pool(name="ps1", bufs=4, space="PSUM") as ps1:
            xT = p1.tile([128, CT, N], f32r)
            wv = p1.tile([128, CT, N], f32r)
            nc.sync.dma_start(
                xT[:], xT_d.rearrange("(t p) n -> p t n", p=128)
            )
            nc.sync.dma_start(
                wv[:],
                wqkv_d[:, 2 * C : 3 * C].rearrange("(t p) n -> p t n", p=128),
            )

            # q (m 0..7, 512 cols) and new-k (m 8..15, 1024 cols) projections
            for m in range(2 * QM):
                ncols = NH if m < QM else N
                nch = ncols // 512
                w = wpool.tile([128, CT, 128], f32r, tag="w")
                nc.sync.dma_start(
                    w[:],
                    wqkv_d[:, m * 128 : (m + 1) * 128].rearrange(
                        "(t p) c -> p t c", p=128
                    ),
                )
                psums = [ps1.tile([128, 512], f32, tag="ps1", name=f"ps1_{m}_{j}") for j in range(nch)]
                for ct in range(CT):
                    for j in range(nch):
                        nc.tensor.matmul(
                            psums[j][:],
                            lhsT=w[:, ct, :],
                            rhs=xT[:, ct, j * 512 : (j + 1) * 512],
                            start=(ct == 0),
                            stop=(ct == CT - 1),
                            tile_position=(0, 0),
                        )
                for j in range(nch):
                    if m < QM:
                        nc.vector.tensor_copy(
                            qT[:, m, j * 512 : (j + 1) * 512], psums[j][:]
                        )
                    else:
                        nc.vector.tensor_copy(
                            kTn[:, m - QM, j * 512 : (j + 1) * 512],
                            psums[j][:],
                        )
                if m < QM:
                    # build opposite-half q copies via SBUF->SBUF DMA
                    nc.sync.dma_start(qd2[64:128, m, :], qT[0:64, m, :])
                    nc.sync.dma_start(qd2[0:64, m, :], qT[64:128, m, :])

            # new-V projection in natural orientation: out[l_part, (h d)]
            for nt in range(NT):
                psums = [ps1.tile([128, 512], f32, tag="ps1", name=f"ps1v_{nt}_{j}") for j in range(2)]
                for ct in range(CT):
                    for j in range(2):
                        nc.tensor.matmul(
                            psums[j][:],
                            lhsT=xT[:, ct, nt * 128 : (nt + 1) * 128],
                            rhs=wv[:, ct, j * 512 : (j + 1) * 512],
                            start=(ct == 0),
                            stop=(ct == CT - 1),
                            tile_position=(0, 0),
                        )
                for j in range(2):
                    nc.vector.tensor_copy(
                        vn[:, nt, j * 8 : (j + 1) * 8, 0:D],
                        psums[j][:].rearrange("p (h d) -> p h d", d=D),
                    )

        # ---------------- phase 2: attention per head ----------------
        with tc.tile_pool(name="kc", bufs=2) as kcp, \
             tc.tile_pool(name="vc", bufs=2) as vcp, \
             tc.tile_pool(name="at", bufs=4) as atp, \
             tc.tile_pool(name="nrm", bufs=2) as nrmp, \
             tc.tile_pool(name="sps", bufs=3, space="PSUM") as sps, \
             tc.tile_pool(name="ops", bufs=2, space="PSUM") as ops, \
             tc.tile_pool(name="bps", bufs=2, space="PSUM") as bps:
            for h in range(H):
                hm, hp = h // 2, (h % 2) * 64
                kc = kcp.tile([128, KCT // 2, 128], f32r, tag="kc")
                nc.sync.dma_start(
                    kc[:].rearrange("p t j -> p (t j)"),
                    kT_d[h].rearrange("a d l -> (a d) l"),
                )
                vc = vcp.tile([128, D + 1, KCT], f32r, tag="vc")
                nc.sync.dma_start(vc[:], v_d[h])

                op = ops.tile([128, NH], f32, tag="op")
                for jt in range(LT):
                    if jt < KCT // 2:
                        lhsT, base = kc[0:64, jt, :], 0
                    elif jt < KCT:
                        lhsT, base = kc[64:128, jt - KCT // 2, :], 64
                    else:
                        j = jt - KCT
                        lhsT, base = kTn[hp : hp + 64, hm, j * 128 : (j + 1) * 128], hp
                    rq = (qT if base == hp else qd2)[base : base + 64, hm, :]
                    sp = sps.tile([128, NH], f32, tag="sp")
                    nc.tensor.matmul(
                        sp[:], lhsT=lhsT, rhs=rq, start=True, stop=True,
                        tile_position=(base, 0),
                    )
                    at = atp.tile([128, NH], f32r, tag="at")
                    nc.scalar.activation(at[:], sp[:], AF.Exp, scale=SCALE)
                    vt = vc[:, :, jt] if jt < KCT else vn[:, jt - KCT, h, :]
                    nc.tensor.matmul(
                        op[0:65, :], lhsT=vt, rhs=at[:],
                        start=(jt == 0), stop=(jt == LT - 1),
                        tile_position=(0, 0), skip_group_check=True,
                    )

                rc = nrmp.tile([1, NH], f32r, tag="rc")
                with nc.allow_low_precision(reason="fp32r reciprocal is fp32-width"):
                    nc.vector.reciprocal(rc[:], op[64:65, :])
                bp = bps.tile([64, NH], f32, tag="bp")
                nc.tensor.matmul(
                    bp[:], lhsT=ones[:], rhs=rc[:], start=True, stop=True,
                    tile_position=(0, 0),
                )
                bc = nrmp.tile([64, NH], f32r, tag="bc")
                nc.vector.tensor_copy(bc[:], bp[:])
                if hp == 0:
                    nc.vector.tensor_tensor(
                        aoT[0:64, hm, :], op[0:64, :], bc[:], op=MUL,
                    )
                else:
                    tmp = nrmp.tile([64, NH], f32r, tag="tmp")
                    nc.vector.tensor_tensor(
                        tmp[:], op[0:64, :], bc[:], op=MUL
                    )
                    nc.sync.dma_start(aoT[64:128, hm, :], tmp[:])

        # ---------------- phase 3: output projection ----------------
        with tc.tile_pool(name="w3", bufs=3) as wp3, \
             tc.tile_pool(name="ysb", bufs=2) as ysb, \
             tc.tile_pool(name="yps", bufs=2, space="PSUM") as yps:
            for m in range(QM):
                w = wp3.tile([128, CT, 128], f32r, tag="w3")
                nc.sync.dma_start(
                    w[:],
                    wproj_d[:, m * 128 : (m + 1) * 128].rearrange(
                        "(t p) c -> p t c", p=128
                    ),
                )
                yp = yps.tile([128, NH], f32, tag="yp")
                for ct in range(CT):
                    nc.tensor.matmul(
                        yp[:], lhsT=w[:, ct, :], rhs=aoT[:, ct, :],
                        start=(ct == 0), stop=(ct == CT - 1), tile_position=(0, 0),
                    )
                y = ysb.tile([128, NH], f32, tag="y")
                nc.vector.tensor_scalar_add(y[:], yp[:], bias[:, m : m + 1])
                nc.sync.dma_start(yT_d[m * 128 : (m + 1) * 128, :], y[:])

    nc.compile()
    return nc


def get_nc():
    if "nc" not in _CACHE:
        _CACHE["nc"] = _build()
    return _CACHE["nc"]


def make_inputs(x, kv_cache, w_qkv, w_proj, b_proj):
    """Host-side shard + layout prep.  Returns list of 8 input maps."""
    x = np.ascontiguousarray(x, dtype=np.float32)
    kv_cache = np.ascontiguousarray(kv_cache, dtype=np.float32)
    w_qkv = np.ascontiguousarray(w_qkv, dtype=np.float32)
    w_proj = np.ascontiguousarray(w_proj, dtype=np.float32)
    b_proj = np.ascontiguousarray(b_proj, dtype=np.float32)

    bias_h = np.ascontiguousarray(b_proj.reshape(QM, 128).T)
    in_maps = []
    for core in range(NCORES):
        b, half = core // 2, core % 2
        xb = x[b]                                    # [N, C]
        own = xb[half * NH : (half + 1) * NH]
        other = xb[(1 - half) * NH : (2 - half) * NH]
        xrot = np.concatenate([own, other], axis=0)  # rotated: own half first
        xT = np.ascontiguousarray(xrot.T)            # [C, N]
        kT = np.ascontiguousarray(
            kv_cache[0, b].reshape(H, 2, LC // 2, D).transpose(0, 1, 3, 2)
        )                                            # [H, 2, D, LC//2]
        v = np.empty((H, 128, D + 1, KCT), dtype=np.float32)
        v[:, :, :D, :] = kv_cache[1, b].reshape(H, KCT, 128, D).transpose(0, 2, 3, 1)
        v[:, :, D, :] = 1.0
        in_maps.append(
            {
                "xT": xT,
                "kT": kT,
                "v": v,
                "wqkv": w_qkv,
                "wproj": w_proj,
                "bias": bias_h,
            }
        )
    return in_maps


def assemble(results):
    y = np.empty((B, N, C), dtype=np.float32)
    for core in range(NCORES):
        b, half = core // 2, core % 2
        y[b, half * NH : (half + 1) * NH] = results[core]["yT"].T
    return y


def kernel(x, kv_cache, w_qkv, w_proj, b_proj):
    from concourse.bass_utils import run_bass_kernel_spmd

    nc = get_nc()
    in_maps = make_inputs(x, kv_cache, w_qkv, w_proj, b_proj)
    res = run_bass_kernel_spmd(nc, in_maps, list(range(NCORES)))
    return assemble(res.results)


# revision 16
# speedup vs baseline: 63.1620x; 63.1620x over previous
"""Attention w/ KV cache on 8 trn2 NeuronCores.

Sharding: core i handles (batch b = i//2, query-half = i%2).  Each core
computes full attention for 512 query rows of one batch against that
batch's 5120 keys (4096 cache + 1024 new).  No collectives: the output
projection contracts over all heads, which every core has for its rows.

All tensors are kept feature-major ("transposed") on chip so that every
matmul consumes operands in its natural orientation; the required
transposes (x^T, cache-K^T, partition-major V) are done host-side in
numpy inside kernel().  Matmuls run in float32r (full PE rate for free
dim >= 256), accumulation in fp32 PSUM.
"""

import sys

sys.path.insert(0, "/opt/trn_rl_repo")

import numpy as np

B, N, C, H, D = 4, 1024, 1024, 16, 64
LC = 4096          # cached keys
L = LC + N         # total keys
NH = N // 2        # query rows per core
NCORES = 8
SCALE = float(D) ** -0.5
CT = C // 128      # contraction tiles (8)
QM = C // 128      # q output col-tiles (8)
NT = N // 128      # new-token l-tiles (8)
LT = L // 128      # total l-tiles (40)
KCT = LC // 128    # cached l-tiles (32)

_CACHE = {}


def _build(reps: int = 1, noact: bool = False, nomm: bool = False, onekv: bool = False):
    import concourse.mybir as mybir
    import concourse.tile as tile
    from concourse import bacc
    from contextlib import ExitStack

    f32 = mybir.dt.float32
    f32r = mybir.dt.float32r
    AF = mybir.ActivationFunctionType
    MUL = mybir.AluOpType.mult

    nc = bacc.Bacc("TRN2", target_bir_lowering=False, debug=False)

    xT_d = nc.dram_tensor("xT", [C, N], f32r, kind="ExternalInput").ap()
    kT_d = nc.dram_tensor("kT", [H, 2, D, LC // 2], f32r, kind="ExternalInput").ap()
    v_d = nc.dram_tensor("v", [H, 128, D + 1, KCT], f32r, kind="ExternalInput").ap()
    wqkv_d = nc.dram_tensor("wqkv", [C, 3 * C], f32r, kind="ExternalInput").ap()
    wproj_d = nc.dram_tensor("wproj", [C, C], f32r, kind="ExternalInput").ap()
    bias_d = nc.dram_tensor("bias", [128, QM], f32, kind="ExternalInput").ap()
    yT_d = nc.dram_tensor("yT", [C, NH], f32, kind="ExternalOutput").ap()

    with tile.TileContext(nc) as tc:
      for _rep in range(reps):
       with ExitStack() as es:
        pp = es.enter_context(tc.tile_pool(name="persist", bufs=1))
        qT = pp.tile([128, QM, NH], f32r)            # q^T, head h at (64*(h%2), h//2)
        qd2 = pp.tile([128, QM, NH], f32r)           # q^T copies on opposite halves
        kTn = pp.tile([128, QM, N], f32r)            # new-K^T, same packing
        vn = pp.tile([128, NT, H, D + 1], f32r)      # new-V + ones col, l-partition-major
        aoT = pp.tile([128, CT, NH], f32r)           # normalized attention out^T
        ones = pp.tile([1, 64], f32r)
        bias = pp.tile([128, QM], f32)
        ones32 = pp.tile([128, 128], f32)

        nc.vector.memset(ones32[:], 1.0)
        if noact:
            atc32 = pp.tile([128, NH], f32)
            nc.vector.memset(atc32[:], 1.0)
            atconst = pp.tile([128, NH], f32r)
            nc.vector.tensor_copy(atconst[:], atc32[:])
        nc.vector.tensor_copy(ones[:], ones32[0:1, 0:64])
        nc.vector.tensor_copy(
            vn[:, :, :, D], ones32[:].rearrange("p (a b) -> p a b", a=NT)
        )
        nc.sync.dma_start(bias[:], bias_d[:])

        # ---------------- phase 1: projections ----------------
        with tc.tile_pool(name="ph1", bufs=1) as p1, \
             tc.tile_pool(name="w1", bufs=3) as wpool, \
             tc.tile_pool(name="ps1", bufs=4, space="PSUM") as ps1:
            xT = p1.tile([128, CT, N], f32r)
            wv = p1.tile([128, CT, N], f32r)
            nc.sync.dma_start(
                xT[:], xT_d.rearrange("(t p) n -> p t n", p=128)
            )
            nc.sync.dma_start(
                wv[:],
                wqkv_d[:, 2 * C : 3 * C].rearrange("(t p) n -> p t n", p=128),
            )

            # q (m 0..7, 512 cols) and new-k (m 8..15, 1024 cols) projections
            for m in range(2 * QM):
                ncols = NH if m < QM else N
                nch = ncols // 512
                w = wpool.tile([128, CT, 128], f32r, tag="w")
                nc.sync.dma_start(
                    w[:],
                    wqkv_d[:, m * 128 : (m + 1) * 128].rearrange(
                        "(t p) c -> p t c", p=128
                    ),
                )
                psums = [ps1.tile([128, 512], f32, tag="ps1", name=f"ps1_{m}_{j}") for j in range(nch)]
                for ct in range(CT):
                    for j in range(nch):
                        nc.tensor.matmul(
                            psums[j][:],
                            lhsT=w[:, ct, :],
                            rhs=xT[:, ct, j * 512 : (j + 1) * 512],
                            start=(ct == 0),
                            stop=(ct == CT - 1),
                            tile_position=(0, 0),
                        )
                for j in range(nch):
                    if m < QM:
                        nc.vector.tensor_copy(
                            qT[:, m, j * 512 : (j + 1) * 512], psums[j][:]
                        )
                    else:
                        nc.vector.tensor_copy(
                            kTn[:, m - QM, j * 512 : (j + 1) * 512],
                            psums[j][:],
                        )
                if m < QM:
                    # build opposite-half q copies via SBUF->SBUF DMA
                    nc.sync.dma_start(qd2[64:128, m, :], qT[0:64, m, :])
                    nc.sync.dma_start(qd2[0:64, m, :], qT[64:128, m, :])

            # new-V projection in natural orientation: out[l_part, (h d)]
            for nt in range(NT):
                psums = [ps1.tile([128, 512], f32, tag="ps1", name=f"ps1v_{nt}_{j}") for j in range(2)]
                for ct in range(CT):
                    for j in range(2):
                        nc.tensor.matmul(
                            psums[j][:],
                            lhsT=xT[:, ct, nt * 128 : (nt + 1) * 128],
                            rhs=wv[:, ct, j * 512 : (j + 1) * 512],
                            start=(ct == 0),
                            stop=(ct == CT - 1),
                            tile_position=(0, 0),
                        )
                for j in range(2):
                    nc.vector.tensor_copy(
                        vn[:, nt, j * 8 : (j + 1) * 8, 0:D],
                        psums[j][:].rearrange("p (h d) -> p h d", d=D),
                    )

        # ---------------- phase 2: attention per head ----------------
        with tc.tile_pool(name="kc", bufs=2) as kcp, \
             tc.tile_pool(name="vc", bufs=2) as vcp, \
             tc.tile_pool(name="at", bufs=6) as atp, \
             tc.tile_pool(name="nrm", bufs=2) as nrmp, \
             tc.tile_pool(name="sps", bufs=3, space="PSUM") as sps, \
             tc.tile_pool(name="ops", bufs=2, space="PSUM") as ops, \
             tc.tile_pool(name="bps", bufs=2, space="PSUM") as bps:
            for h in range(H):
                hm, hp = h // 2, (h % 2) * 64
                if not onekv or h == 0:
                    kc = kcp.tile([128, KCT // 2, 128], f32r, tag="kc", name=f"kc{h}")
                    nc.sync.dma_start(
                        kc[:].rearrange("p t j -> p (t j)"),
                        kT_d[h].rearrange("a d l -> (a d) l"),
                    )
                    vc = vcp.tile([128, D + 1, KCT], f32r, tag="vc", name=f"vc{h}")
                    nc.sync.dma_start(vc[:], v_d[h])
                if nomm:
                    continue

                op = ops.tile([128, NH], f32, tag="op")
                jt_order = list(range(LT))
                for ji, jt in enumerate(jt_order):
                    if jt < KCT // 2:
                        lhsT, base = kc[0:64, jt, :], 0
                    elif jt < KCT:
                        lhsT, base = kc[64:128, jt - KCT // 2, :], 64
                    else:
                        j = jt - KCT
                        lhsT, base = kTn[hp : hp + 64, hm, j * 128 : (j + 1) * 128], hp
                    rq = (qT if base == hp else qd2)[base : base + 64, hm, :]
                    sp = sps.tile([128, NH], f32, tag="sp")
                    nc.tensor.matmul(
                        sp[:], lhsT=lhsT, rhs=rq, start=True, stop=True,
                        tile_position=(base, 0),
                    )
                    if noact:
                        at = atconst
                    else:
                        at = atp.tile([128, NH], f32r, tag="at")
                        nc.scalar.activation(at[:], sp[:], AF.Exp, scale=SCALE)
                    vt = vc[:, :, jt] if jt < KCT else vn[:, jt - KCT, h, :]
                    nc.tensor.matmul(
                        op[0:65, :], lhsT=vt, rhs=at[:],
                        start=(ji == 0), stop=(ji == len(jt_order) - 1),
                        tile_position=(0, 0), skip_group_check=True,
                    )

                rc = nrmp.tile([1, NH], f32r, tag="rc")
                with nc.allow_low_precision(reason="fp32r reciprocal is fp32-width"):
                    nc.vector.reciprocal(rc[:], op[64:65, :])
                bp = bps.tile([64, NH], f32, tag="bp")
                nc.tensor.matmul(
                    bp[:], lhsT=ones[:], rhs=rc[:], start=True, stop=True,
                    tile_position=(0, 0),
                )
                bc = nrmp.tile([64, NH], f32r, tag="bc")
                nc.vector.tensor_copy(bc[:], bp[:])
                if hp == 0:
                    nc.vector.tensor_tensor(
                        aoT[0:64, hm, :], op[0:64, :], bc[:], op=MUL,
                    )
                else:
                    tmp = nrmp.tile([64, NH], f32r, tag="tmp")
                    nc.vector.tensor_tensor(
                        tmp[:], op[0:64, :], bc[:], op=MUL
                    )
                    nc.sync.dma_start(aoT[64:128, hm, :], tmp[:])

        # ---------------- phase 3: output projection ----------------
        with tc.tile_pool(name="w3", bufs=3) as wp3, \
             tc.tile_pool(name="ysb", bufs=2) as ysb, \
             tc.tile_pool(name="yps", bufs=2, space="PSUM") as yps:
            for m in range(QM if not nomm else 0):
                w = wp3.tile([128, CT, 128], f32r, tag="w3")
                nc.sync.dma_start(
                    w[:],
                    wproj_d[:, m * 128 : (m + 1) * 128].rearrange(
                        "(t p) c -> p t c", p=128
                    ),
                )
                yp = yps.tile([128, NH], f32, tag="yp")
                for ct in range(CT):
                    nc.tensor.matmul(
                        yp[:], lhsT=w[:, ct, :], rhs=aoT[:, ct, :],
                        start=(ct == 0), stop=(ct == CT - 1), tile_position=(0, 0),
                    )
                y = ysb.tile([128, NH], f32, tag="y")
                nc.vector.tensor_scalar_add(y[:], yp[:], bias[:, m : m + 1])
                nc.sync.dma_start(yT_d[m * 128 : (m + 1) * 128, :], y[:])

    nc.compile()
    return nc


def get_nc(reps: int = 1, noact: bool = False, nomm: bool = False, onekv: bool = False):
    key = f"nc{reps}_{noact}_{nomm}_{onekv}"
    if key not in _CACHE:
        _CACHE[key] = _build(reps, noact, nomm, onekv)
    return _CACHE[key]


def make_inputs(x, kv_cache, w_qkv, w_proj, b_proj):
    """Host-side shard + layout prep.  Returns list of 8 input maps."""
    x = np.ascontiguousarray(x, dtype=np.float32)
    kv_cache = np.ascontiguousarray(kv_cache, dtype=np.float32)
    w_qkv = np.ascontiguousarray(w_qkv, dtype=np.float32)
    w_proj = np.ascontiguousarray(w_proj, dtype=np.float32)
    b_proj = np.ascontiguousarray(b_proj, dtype=np.float32)

    bias_h = np.ascontiguousarray(b_proj.reshape(QM, 128).T)
    in_maps = []
    for core in range(NCORES):
        b, half = core // 2, core % 2
        xb = x[b]                                    # [N, C]
        own = xb[half * NH : (half + 1) * NH]
        other = xb[(1 - half) * NH : (2 - half) * NH]
        xrot = np.concatenate([own, other], axis=0)  # rotated: own half first
        xT = np.ascontiguousarray(xrot.T)            # [C, N]
        kT = np.ascontiguousarray(
            kv_cache[0, b].reshape(H, 2, LC // 2, D).transpose(0, 1, 3, 2)
        )                                            # [H, 2, D, LC//2]
        v = np.empty((H, 128, D + 1, KCT), dtype=np.float32)
        v[:, :, :D, :] = kv_cache[1, b].reshape(H, KCT, 128, D).transpose(0, 2, 3, 1)
        v[:, :, D, :] = 1.0
        in_maps.append(
            {
                "xT": xT,
                "kT": kT,
                "v": v,
                "wqkv": w_qkv,
                "wproj": w_proj,
                "bias": bias_h,
            }
        )
    return in_maps


def assemble(results):
    y = np.empty((B, N, C), dtype=np.float32)
    for core in range(NCORES):
        b, half = core // 2, core % 2
        y[b, half * NH : (half + 1) * NH] = results[core]["yT"].T
    return y


def kernel(x, kv_cache, w_qkv, w_proj, b_proj):
    from concourse.bass_utils import run_bass_kernel_spmd

    nc = get_nc()
    in_maps = make_inputs(x, kv_cache, w_qkv, w_proj, b_proj)
    res = run_bass_kernel_spmd(nc, in_maps, list(range(NCORES)))
    return assemble(res.results)
